# revision 16
# baseline (speedup 1.0000x reference)
"""DeepSeek sparse attention (single-query, MQA low-rank KV) on 8 trn2 cores.

Strategy (data-parallel: batch b -> core b):
  Launch 1 (device): full-S K_down pass in bf16 (noisy, ~0.2 score-units err)
      -> fp8-quantize -> indexer scores vs q_idx  -> noisy scores out.
  Host: top-k certain/band split (margin 384 ranks); band rows rescored
      bit-exactly vs the reference via jax-CPU slice gemm (XLA slice gemm is
      bitwise-identical to the full gemm rows, verified); exact top-k set.
  Launch 2 (device): host-gathered x_sel -> K_sel/V_sel down-proj in f32r,
      per-head up-proj + attention + out-proj in f32r.

Shapes hardcoded: B=8, S=8192, D=2048, H=16, dh=128, L=512, k=2048.
"""
import os
import numpy as np
import ml_dtypes

import concourse.bacc as bacc
import concourse.tile as tile
import concourse.mybir as mybir
from concourse import masks
from concourse.bass_utils import run_bass_kernel_spmd

BF16 = ml_dtypes.bfloat16
dt = mybir.dt

B, S, D = 8, 8192, 2048
H, DH, L = 16, 128, 512
TOPK = 2048
MARGIN = 384
NCORES = 8
RSQ = float(1.0 / np.sqrt(np.float32(DH)))  # 1/sqrt(128)

_STATE = {}
LAST_EXEC = {}
LAST_IN1 = []
LAST_IN2 = []


# ---------------------------------------------------------------- launch 1
def _build_l1():
    nc = bacc.Bacc("TRN2", target_bir_lowering=False, debug=False,
                   num_devices=NCORES)
    xT8 = nc.dram_tensor("xT8", [D, S], dt.bfloat16, kind="ExternalInput").ap()
    wdk8 = nc.dram_tensor("wdk8", [D, L], dt.bfloat16, kind="ExternalInput").ap()
    qxT8 = nc.dram_tensor("qxT8", [L, B], dt.bfloat16, kind="ExternalInput").ap()
    bkdT = nc.dram_tensor("bkdT", [128, 4], dt.float32, kind="ExternalInput").ap()
    scores = nc.dram_tensor("scores", [B, S], dt.float32, kind="ExternalOutput").ap()

    NSB = S // 512       # 16 s-blocks
    ND = D // 128        # 16 d-chunks
    NL = L // 128        # 4 l-tiles

    with tile.TileContext(nc) as tc:
        with (
            tc.tile_pool(name="wpool", bufs=1) as wpool,
            tc.tile_pool(name="xpool", bufs=2) as xpool,
            tc.tile_pool(name="f8pool", bufs=4) as f8pool,
            tc.tile_pool(name="kps", bufs=5, space="PSUM") as kps,
            tc.tile_pool(name="sps", bufs=2, space="PSUM") as sps,
        ):
            wd = wpool.tile([128, ND * L], dt.bfloat16)
            for c in range(ND):
                nc.sync.dma_start(wd[:, c * L:(c + 1) * L],
                                  wdk8[c * 128:(c + 1) * 128, :])
            qx = wpool.tile([128, NL * B], dt.bfloat16)
            for lt in range(NL):
                nc.sync.dma_start(qx[:, lt * B:(lt + 1) * B],
                                  qxT8[lt * 128:(lt + 1) * 128, :])
            bkd = wpool.tile([128, 4], dt.float32)
            nc.sync.dma_start(bkd[:], bkdT)
            k8b = wpool.tile([128, NL * S], dt.bfloat16)   # K8 as bf16, [l, s]
            ssb = wpool.tile([B, S], dt.float32)

            for sb in range(NSB):
                slab = xpool.tile([128, ND * 512], dt.bfloat16)
                for c in range(ND):
                    nc.sync.dma_start(
                        slab[:, c * 512:(c + 1) * 512],
                        xT8[c * 128:(c + 1) * 128, sb * 512:(sb + 1) * 512])
                for lt in range(NL):
                    pk = kps.tile([128, 512], dt.float32)
                    for d in range(ND):
                        nc.tensor.matmul(
                            pk[:],
                            wd[:, d * L + lt * 128: d * L + lt * 128 + 128],
                            slab[:, d * 512:(d + 1) * 512],
                            start=(d == 0), stop=(d == ND - 1))
                    f8 = f8pool.tile([128, 512], dt.float8e4)
                    # q8(K_down + bias): bias per-partition, cast f32->fp8
                    nc.vector.tensor_scalar_add(f8[:], pk[:], bkd[:, lt:lt + 1])
                    # fp8 -> bf16 (exact embedding) for the scoring matmuls
                    nc.scalar.copy(
                        k8b[:, lt * S + sb * 512: lt * S + sb * 512 + 512],
                        f8[:])

            for sb in range(NSB):
                ps = sps.tile([B, 512], dt.float32)
                for lt in range(NL):
                    nc.tensor.matmul(
                        ps[:],
                        qx[:, lt * B:(lt + 1) * B],
                        k8b[:, lt * S + sb * 512: lt * S + sb * 512 + 512],
                        start=(lt == 0), stop=(lt == NL - 1))
                nc.scalar.copy(ssb[:, sb * 512:(sb + 1) * 512], ps[:])
            nc.sync.dma_start(scores, ssb[:])
    nc.compile()
    return nc


# ---------------------------------------------------------------- launch 2
def _build_l2():
    nc = bacc.Bacc("TRN2", target_bir_lowering=False, debug=False,
                   num_devices=NCORES)
    f32r = dt.float32r
    xselT = nc.dram_tensor("xselT", [D, TOPK], f32r, kind="ExternalInput").ap()
    wkvd = nc.dram_tensor("wkvd", [D, 2 * L], f32r, kind="ExternalInput").ap()
    wktup = nc.dram_tensor("wktup", [D, L], f32r, kind="ExternalInput").ap()
    wvup = nc.dram_tensor("wvup", [L, D], f32r, kind="ExternalInput").ap()
    wout = nc.dram_tensor("wout", [D, D], f32r, kind="ExternalInput").ap()
    qth = nc.dram_tensor("qth", [128, H], f32r, kind="ExternalInput").ap()
    bkdT = nc.dram_tensor("bkdT", [128, 4], dt.float32, kind="ExternalInput").ap()
    bvdr = nc.dram_tensor("bvdr", [128, 512], dt.float32, kind="ExternalInput").ap()
    bvu = nc.dram_tensor("bvu", [DH, H], dt.float32, kind="ExternalInput").ap()
    boutr = nc.dram_tensor("boutr", [1, D], dt.float32, kind="ExternalInput").ap()
    outr = nc.dram_tensor("outr", [1, D], dt.float32, kind="ExternalOutput").ap()

    ND = D // 128        # 16
    NL = L // 128        # 4
    NKB = TOPK // 256    # 8 half-k-blocks (256 wide)
    NKT = TOPK // 128    # 16 k-tiles

    with tile.TileContext(nc) as tc:
        with tc.tile_pool(name="top", bufs=1) as top:
            bkd = top.tile([128, 4], dt.float32)
            nc.sync.dma_start(bkd[:], bkdT)
            bvdrep = top.tile([128, 512], dt.float32)
            nc.sync.dma_start(bvdrep[:], bvdr)
            qthh = top.tile([128, H], f32r)
            nc.sync.dma_start(qthh[:], qth)
            ident = top.tile([128, 128], dt.float32)
            masks.make_identity(nc, ident[:])
            ksT = top.tile([128, NL * TOPK], f32r)         # [l, k] 4 MiB
            vs = top.tile([128, NKT * L], f32r)            # [k, l] 4 MiB

            # ---- phase KV: K_selT (layout B) + V_sel (layout A)
            with (
                tc.tile_pool(name="wkvp", bufs=1) as wkvp,
                tc.tile_pool(name="xpool", bufs=2) as xpool,
                tc.tile_pool(name="kps", bufs=3, space="PSUM") as kps,
            ):
                wkv = wkvp.tile([128, ND * 2 * L], f32r)
                for c in range(ND):
                    nc.sync.dma_start(wkv[:, c * 2 * L:(c + 1) * 2 * L],
                                      wkvd[c * 128:(c + 1) * 128, :])
                for kb in range(NKB):
                    slab = xpool.tile([128, ND * 256], f32r)
                    for c in range(ND):
                        nc.sync.dma_start(
                            slab[:, c * 256:(c + 1) * 256],
                            xselT[c * 128:(c + 1) * 128,
                                  kb * 256:(kb + 1) * 256])
                    for lt in range(NL):
                        pk = kps.tile([128, 256], dt.float32, tag="pk")
                        for d in range(ND):
                            nc.tensor.matmul(
                                pk[:],
                                wkv[:, d * 2 * L + lt * 128:
                                    d * 2 * L + lt * 128 + 128],
                                slab[:, d * 256:(d + 1) * 256],
                                start=(d == 0), stop=(d == ND - 1))
                        nc.vector.tensor_scalar_add(
                            ksT[:, lt * TOPK + kb * 256:
                                lt * TOPK + kb * 256 + 256],
                            pk[:], bkd[:, lt:lt + 1])
                    for kt in range(2):
                        t = kb * 2 + kt
                        pv = kps.tile([128, 512], dt.float32, tag="pv")
                        for d in range(ND):
                            nc.tensor.matmul(
                                pv[:],
                                slab[:, d * 256 + kt * 128:
                                     d * 256 + kt * 128 + 128],
                                wkv[:, d * 2 * L + L: d * 2 * L + 2 * L],
                                start=(d == 0), stop=(d == ND - 1))
                        nc.vector.tensor_add(
                            vs[:, t * L:(t + 1) * L], pv[:], bvdrep[:])

            # ---- qhT[l, h] = sum_dh Wk_upT[hd, l] * qT_heads[hd, h] per head
            # f32r ISA requires even moving-N: compute [l-tile, 16] blocks
            # against ALL heads (rhs = qthh [128,16]) and extract column h.
            qhT = top.tile([128, NL * H], f32r)
            with (
                tc.tile_pool(name="wkp", bufs=2) as wkp,
                tc.tile_pool(name="qps", bufs=2, space="PSUM") as qps,
            ):
                for h in range(H):
                    wkb = wkp.tile([128, L], f32r, tag="wkb")
                    nc.sync.dma_start(wkb[:], wktup[h * 128:(h + 1) * 128, :])
                    for lc in range(NL):
                        pqh = qps.tile([128, H], dt.float32, tag="pqh")
                        nc.tensor.matmul(
                            pqh[:],
                            wkb[:, lc * 128:(lc + 1) * 128],
                            qthh[:],
                            start=True, stop=True)
                        nc.scalar.copy(
                            qhT[:, lc * H + h: lc * H + h + 1],
                            pqh[:, h:h + 1])

            # ---- logits + softmax
            attn2 = top.tile([H, TOPK], dt.float32)
            with (
                tc.tile_pool(name="lpool", bufs=1, space="PSUM") as lpool,
                tc.tile_pool(name="smx", bufs=1) as smx,
            ):
                lps = lpool.tile([128, TOPK], dt.float32)
                for lc in range(NL):
                    for nb in range(4):
                        nc.tensor.matmul(
                            lps[:H, nb * 512:(nb + 1) * 512],
                            qhT[:, lc * H:(lc + 1) * H],
                            ksT[:, lc * TOPK + nb * 512:
                                lc * TOPK + nb * 512 + 512],
                            start=(lc == 0), stop=(lc == NL - 1))
                mx = smx.tile([H, 1], dt.float32)
                nc.vector.reduce_max(mx[:], lps[:H, :],
                                     axis=mybir.AxisListType.X)
                nmx = smx.tile([H, 1], dt.float32)
                nc.vector.tensor_scalar_mul(nmx[:], mx[:], -RSQ)
                attn = smx.tile([H, TOPK], dt.float32)
                den = smx.tile([H, 1], dt.float32)
                nc.scalar.activation(attn[:], lps[:H, :],
                                     mybir.ActivationFunctionType.Exp,
                                     bias=nmx[:], scale=RSQ, accum_out=den[:])
                rden = smx.tile([H, 1], dt.float32)
                nc.vector.reciprocal(rden[:], den[:])
                nc.vector.tensor_scalar_mul(attn2[:], attn[:], rden[:])

            # ---- attnT, m, o, out
            with (
                tc.tile_pool(name="wvp", bufs=2) as wvp,
                tc.tile_pool(name="wop", bufs=3) as wop,
                tc.tile_pool(name="tp", bufs=2, space="PSUM") as tp,
                tc.tile_pool(name="ap", bufs=1, space="PSUM") as ap,
            ):
                attnT = top.tile([128, NKT * H], f32r)
                for t in range(NKT):
                    pt = tp.tile([128, H], dt.float32, tag="tp")
                    nc.tensor.matmul(pt[:], attn2[:, t * 128:(t + 1) * 128],
                                     ident[:H, :H], is_transpose=True)
                    nc.scalar.copy(attnT[:, t * H:(t + 1) * H], pt[:])

                mps = ap.tile([H, L], dt.float32, tag="acc")
                for t in range(NKT):
                    nc.tensor.matmul(mps[:], attnT[:, t * H:(t + 1) * H],
                                     vs[:, t * L:(t + 1) * L],
                                     start=(t == 0), stop=(t == NKT - 1))
                m_sb = top.tile([H, L], dt.float32)
                nc.scalar.copy(m_sb[:], mps[:])
                mT = top.tile([128, NL * H], f32r)
                for lc in range(NL):
                    pmt = tp.tile([128, H], dt.float32, tag="tp")
                    nc.tensor.matmul(pmt[:], m_sb[:, lc * 128:(lc + 1) * 128],
                                     ident[:H, :H], is_transpose=True)
                    nc.scalar.copy(mT[:, lc * H:(lc + 1) * H], pmt[:])

                # oT[dh, h] = sum_lc Wv_up[lc-chunk, h-block].T @ mT[:, lc, h]
                # even-N fix: rhs = all-head mT chunk [128, 16]; accumulate
                # over lc per h (col h valid, others garbage), extract col h.
                wv4 = wvp.tile([128, NL * D], f32r, tag="wv4")
                for lc in range(NL):
                    nc.sync.dma_start(wv4[:, lc * D:(lc + 1) * D],
                                      wvup[lc * 128:(lc + 1) * 128, :])
                bvui = wvp.tile([128, H], dt.float32, tag="bvui")
                nc.sync.dma_start(bvui[:], bvu)
                # oTz: even columns hold oT (+bv_up), odd columns zero, so the
                # out-proj can use even-width [128, 2] stationary slices.
                oTz = wvp.tile([128, 2 * H], f32r, tag="oTz")
                zf = wvp.tile([128, 2 * H], dt.float32, tag="zf")
                nc.vector.memset(zf[:], 0.0)
                nc.vector.tensor_copy(oTz[:], zf[:])
                for h in range(H):
                    poh = ap.tile([128, H], dt.float32, tag="acc2")
                    for lc in range(NL):
                        nc.tensor.matmul(
                            poh[:],
                            wv4[:, lc * D + h * DH: lc * D + (h + 1) * DH],
                            mT[:, lc * H:(lc + 1) * H],
                            start=(lc == 0), stop=(lc == NL - 1))
                    nc.vector.tensor_add(
                        oTz[:, 2 * h:2 * h + 1], poh[:, h:h + 1],
                        bvui[:, h:h + 1])

                # out = o_flat @ Wout + bout
                bouti = wvp.tile([1, D], dt.float32, tag="bouti")
                nc.sync.dma_start(bouti[:], boutr)
                out_sb = wvp.tile([1, D], dt.float32, tag="out_sb")
                for nb in range(4):
                    # M=2 (even) stationary: col 0 real o-chunk, col 1 zeros
                    pout = tp.tile([2, 512], dt.float32, tag="tp")
                    for dc in range(ND):
                        wob = wop.tile([128, 512], f32r, tag="wob")
                        nc.sync.dma_start(
                            wob[:], wout[dc * 128:(dc + 1) * 128,
                                         nb * 512:(nb + 1) * 512])
                        nc.tensor.matmul(pout[:], oTz[:, 2 * dc:2 * dc + 2],
                                         wob[:],
                                         start=(dc == 0), stop=(dc == ND - 1))
                    nc.vector.tensor_add(
                        out_sb[:, nb * 512:(nb + 1) * 512], pout[:1, :],
                        bouti[:, nb * 512:(nb + 1) * 512])
                nc.sync.dma_start(outr, out_sb[:])
    nc.compile()
    return nc


# ---------------------------------------------------------------- timing
def time_launch(nc, in_maps, iters=20):
    """Measure per-execution HW time of a compiled launch: build the sharded
    PJRT executable once, keep inputs device-resident, pipeline `iters`
    executions and average. Donated zero output buffers are refreshed per
    call (tiny)."""
    import time as _time
    import jax
    from jax.sharding import Mesh, PartitionSpec, NamedSharding
    from jax.experimental.shard_map import shard_map
    from concourse import bass2jax

    bass2jax.install_neuronx_cc_hook()
    pname = nc.partition_id_tensor.name if nc.partition_id_tensor else None
    in_names, out_names, out_avals = [], [], []
    for alloc in nc.m.functions[0].allocations:
        if not isinstance(alloc, mybir.MemoryLocationSet):
            continue
        name = alloc.memorylocations[0].name
        if alloc.kind == "ExternalInput":
            if name != pname:
                in_names.append(name)
        elif alloc.kind == "ExternalOutput":
            out_names.append(name)
            out_avals.append(jax.core.ShapedArray(
                tuple(alloc.tensor_shape), mybir.dt.np(alloc.dtype)))
    n_params = len(in_names)
    all_in = in_names + out_names
    if pname is not None:
        all_in = all_in + [pname]
    donate = tuple(range(n_params, n_params + len(out_names)))

    def _body(*args):
        operands = list(args)
        if pname is not None:
            operands.append(bass2jax.partition_id_tensor())
        outs = bass2jax._bass_exec_p.bind(
            *operands, out_avals=tuple(out_avals), in_names=tuple(all_in),
            out_names=tuple(out_names), lowering_input_output_aliases=(),
            sim_require_finite=True, sim_require_nnan=True, nc=nc)
        return tuple(outs)

    n = len(in_maps)
    devices = jax.devices()[:n]
    mesh = Mesh(np.asarray(devices), ("core",))
    fn = jax.jit(
        shard_map(_body, mesh=mesh,
                  in_specs=(PartitionSpec("core"),) * (n_params + len(out_names)),
                  out_specs=(PartitionSpec("core"),) * len(out_names),
                  check_rep=False),
        donate_argnums=donate, keep_unused=True)
    sh = NamedSharding(mesh, PartitionSpec("core"))
    concat_in = [
        jax.device_put(
            np.concatenate([np.asarray(m[name]) for m in in_maps], axis=0), sh)
        for name in in_names]

    def zeros():
        return [jax.device_put(
            np.zeros((n * av.shape[0], *av.shape[1:]), av.dtype), sh)
            for av in out_avals]

    out = fn(*concat_in, *zeros())
    jax.block_until_ready(out)
    zs = [zeros() for _ in range(iters)]
    jax.block_until_ready(zs)
    t0 = _time.perf_counter()
    outs = [fn(*concat_in, *z) for z in zs]
    jax.block_until_ready(outs)
    t1 = _time.perf_counter()
    return (t1 - t0) / iters * 1e9


def model_time(nc):
    """Cost-model (TimelineSim) estimate in ns for one core."""
    from concourse.timeline_sim import TimelineSim
    return TimelineSim(nc).simulate()


def _q8j(a):
    import jax.numpy as jnp
    return jnp.asarray(a).astype(jnp.float8_e4m3fn).astype(jnp.float32)


def kernel(**inputs):
    import jax
    import jax.numpy as jnp
    cpu = jax.devices("cpu")[0]

    x = np.ascontiguousarray(np.asarray(inputs["x"], dtype=np.float32))
    Wq = np.asarray(inputs["Wq"], dtype=np.float32)
    bq = np.asarray(inputs["bq"], dtype=np.float32)
    Wkv_down = np.asarray(inputs["Wkv_down"], dtype=np.float32)
    bkv_down = np.asarray(inputs["bkv_down"], dtype=np.float32)
    Wq_down = np.asarray(inputs["Wq_down"], dtype=np.float32)
    bq_down = np.asarray(inputs["bq_down"], dtype=np.float32)
    Wkv_up = np.asarray(inputs["Wkv_up"], dtype=np.float32)
    bkv_up = np.asarray(inputs["bkv_up"], dtype=np.float32)
    Wout = np.asarray(inputs["Wout"], dtype=np.float32)
    bout = np.asarray(inputs["bout"], dtype=np.float32)
    k = int(np.asarray(inputs["top_k"]))
    assert k == TOPK, f"kernel hardcoded for top_k={TOPK}, got {k}"

    if "l1" not in _STATE:
        _STATE["l1"] = _build_l1()
    if "l2" not in _STATE:
        _STATE["l2"] = _build_l2()

    trace = False  # NTFF profiling hook unavailable under this axon client

    q_last = x[:, -1, :]                                   # [B, D]
    with jax.default_device(cpu):
        # bit-exact replication of the reference's fp8 indexer query + q
        q_idx = np.asarray(_q8j(q_last) @ _q8j(Wq_down) + _q8j(bq_down))
        q = np.asarray(jnp.asarray(q_last) @ jnp.asarray(Wq)) + bq

    # ---------------- launch 1: noisy full-S scores
    wdk8 = np.ascontiguousarray(Wkv_down[:, :L]).astype(BF16)
    qxT8 = np.ascontiguousarray(q_idx.T).astype(BF16)      # [L, B]
    bkdT = np.ascontiguousarray(bkv_down[:L].reshape(4, 128).T)
    in1 = []
    for c in range(NCORES):
        in1.append({
            "xT8": np.ascontiguousarray(x[c].T).astype(BF16),
            "wdk8": wdk8,
            "qxT8": qxT8,
            "bkdT": bkdT,
        })
    LAST_IN1.clear(); LAST_IN1.extend(in1)
    r1 = run_bass_kernel_spmd(_STATE["l1"], in1, list(range(NCORES)),
                              trace=trace)
    LAST_EXEC["l1"] = r1
    s_noisy = np.stack([r1.results[c]["scores"][c] for c in range(NCORES)])

    # ---------------- host: exact top-k set via band rescore (bit-exact)
    sel_all = []
    with jax.default_device(cpu):
        jWdk = jnp.asarray(Wkv_down[:, :L])
        jbkd = jnp.asarray(bkv_down[:L])
        for b in range(B):
            order = np.argsort(-np.maximum(s_noisy[b], 0.0), kind="stable")
            certain = order[:k - MARGIN]
            band = order[k - MARGIN:k + MARGIN]
            Kb = jnp.asarray(x[b][band]) @ jWdk + jbkd
            sb = np.asarray(jnp.einsum(
                "l,sl->s", jnp.asarray(q_idx[b]),
                Kb.astype(jnp.float8_e4m3fn).astype(jnp.float32)))
            sb = np.maximum(sb, 0.0)
            pick = band[np.argsort(-sb, kind="stable")[:k - len(certain)]]
            sel_all.append(np.concatenate([certain, pick]))

    # ---------------- launch 2: attention over the selected set
    wktup = np.ascontiguousarray(Wkv_up[:, :D].T)          # [D, L]
    wvup = np.ascontiguousarray(Wkv_up[:, D:])             # [L, D]
    bvdr = np.ascontiguousarray(
        np.broadcast_to(bkv_down[L:], (128, 512))).astype(np.float32)
    bvu = np.ascontiguousarray(bkv_up[D:].reshape(H, DH).T)
    boutr = np.ascontiguousarray(bout.reshape(1, D))
    in2 = []
    for c in range(NCORES):
        in2.append({
            "xselT": np.ascontiguousarray(x[c][sel_all[c]].T),
            "wkvd": Wkv_down,
            "wktup": wktup,
            "wvup": wvup,
            "wout": Wout,
            "qth": np.ascontiguousarray(q[c].reshape(H, DH).T),
            "bkdT": bkdT,
            "bvdr": bvdr,
            "bvu": bvu,
            "boutr": boutr,
        })
    LAST_IN2.clear(); LAST_IN2.extend(in2)
    r2 = run_bass_kernel_spmd(_STATE["l2"], in2, list(range(NCORES)),
                              trace=trace)
    LAST_EXEC["l2"] = r2
    out = np.stack([r2.results[c]["outr"][0] for c in range(NCORES)])
    return out.astype(np.float32)


# revision 18
# speedup vs baseline: 438.0441x; 438.0441x over previous
"""DeepSeek sparse attention (single-query, MQA low-rank KV) on 8 trn2 cores.

Strategy (data-parallel: batch b -> core b):
  Launch 1 (device): full-S K_down pass in bf16 (noisy, ~0.2 score-units err)
      -> fp8-quantize -> indexer scores vs q_idx  -> noisy scores out.
  Host: top-k certain/band split (margin 384 ranks); band rows rescored
      bit-exactly vs the reference via jax-CPU slice gemm (XLA slice gemm is
      bitwise-identical to the full gemm rows, verified); exact top-k set.
  Launch 2 (device): host-gathered x_sel -> K_sel/V_sel down-proj in f32r,
      per-head up-proj + attention + out-proj in f32r.

Shapes hardcoded: B=8, S=8192, D=2048, H=16, dh=128, L=512, k=2048.
"""
import os
import numpy as np
import ml_dtypes

import concourse.bacc as bacc
import concourse.tile as tile
import concourse.mybir as mybir
from concourse import masks
from concourse.bass_utils import run_bass_kernel_spmd

BF16 = ml_dtypes.bfloat16
dt = mybir.dt

B, S, D = 8, 8192, 2048
H, DH, L = 16, 128, 512
TOPK = 2048
MARGIN = 384
NCORES = 8
RSQ = float(1.0 / np.sqrt(np.float32(DH)))  # 1/sqrt(128)

_STATE = {}
LAST_EXEC = {}
LAST_IN1 = []
LAST_IN2 = []


# ---------------------------------------------------------------- launch 1
def _build_l1():
    nc = bacc.Bacc("TRN2", target_bir_lowering=False, debug=False,
                   num_devices=NCORES)
    xT8 = nc.dram_tensor("xT8", [D, S], dt.bfloat16, kind="ExternalInput").ap()
    wdk8 = nc.dram_tensor("wdk8", [D, L], dt.bfloat16, kind="ExternalInput").ap()
    qxT8 = nc.dram_tensor("qxT8", [L, B], dt.bfloat16, kind="ExternalInput").ap()
    bkdT = nc.dram_tensor("bkdT", [128, 4], dt.float32, kind="ExternalInput").ap()
    scores = nc.dram_tensor("scores", [B, S], dt.float32, kind="ExternalOutput").ap()

    NSB = S // 512       # 16 s-blocks
    ND = D // 128        # 16 d-chunks
    NL = L // 128        # 4 l-tiles

    with tile.TileContext(nc) as tc:
        with (
            tc.tile_pool(name="wpool", bufs=1) as wpool,
            tc.tile_pool(name="xpool", bufs=2) as xpool,
            tc.tile_pool(name="f8pool", bufs=4) as f8pool,
            tc.tile_pool(name="kps", bufs=5, space="PSUM") as kps,
            tc.tile_pool(name="sps", bufs=2, space="PSUM") as sps,
        ):
            wd = wpool.tile([128, ND * L], dt.bfloat16)
            for c in range(ND):
                nc.sync.dma_start(wd[:, c * L:(c + 1) * L],
                                  wdk8[c * 128:(c + 1) * 128, :])
            qx = wpool.tile([128, NL * B], dt.bfloat16)
            for lt in range(NL):
                nc.sync.dma_start(qx[:, lt * B:(lt + 1) * B],
                                  qxT8[lt * 128:(lt + 1) * 128, :])
            bkd = wpool.tile([128, 4], dt.float32)
            nc.sync.dma_start(bkd[:], bkdT)
            k8b = wpool.tile([128, NL * S], dt.bfloat16)   # K8 as bf16, [l, s]
            ssb = wpool.tile([B, S], dt.float32)

            for sb in range(NSB):
                slab = xpool.tile([128, ND * 512], dt.bfloat16)
                for c in range(ND):
                    nc.sync.dma_start(
                        slab[:, c * 512:(c + 1) * 512],
                        xT8[c * 128:(c + 1) * 128, sb * 512:(sb + 1) * 512])
                for lt in range(NL):
                    pk = kps.tile([128, 512], dt.float32)
                    for d in range(ND):
                        nc.tensor.matmul(
                            pk[:],
                            wd[:, d * L + lt * 128: d * L + lt * 128 + 128],
                            slab[:, d * 512:(d + 1) * 512],
                            start=(d == 0), stop=(d == ND - 1))
                    f8 = f8pool.tile([128, 512], dt.float8e4)
                    # q8(K_down + bias): bias per-partition, cast f32->fp8
                    nc.vector.tensor_scalar_add(f8[:], pk[:], bkd[:, lt:lt + 1])
                    # fp8 -> bf16 (exact embedding) for the scoring matmuls
                    nc.scalar.copy(
                        k8b[:, lt * S + sb * 512: lt * S + sb * 512 + 512],
                        f8[:])

            for sb in range(NSB):
                ps = sps.tile([B, 512], dt.float32)
                for lt in range(NL):
                    nc.tensor.matmul(
                        ps[:],
                        qx[:, lt * B:(lt + 1) * B],
                        k8b[:, lt * S + sb * 512: lt * S + sb * 512 + 512],
                        start=(lt == 0), stop=(lt == NL - 1))
                nc.scalar.copy(ssb[:, sb * 512:(sb + 1) * 512], ps[:])
            nc.sync.dma_start(scores, ssb[:])
    nc.compile()
    return nc


# ---------------------------------------------------------------- launch 2
def _build_l2():
    nc = bacc.Bacc("TRN2", target_bir_lowering=False, debug=False,
                   num_devices=NCORES)
    f32r = dt.float32r
    xselT = nc.dram_tensor("xselT", [D, TOPK], f32r, kind="ExternalInput").ap()
    wkvd = nc.dram_tensor("wkvd", [D, 2 * L], f32r, kind="ExternalInput").ap()
    wktup = nc.dram_tensor("wktup", [D, L], f32r, kind="ExternalInput").ap()
    wvup = nc.dram_tensor("wvup", [L, D], f32r, kind="ExternalInput").ap()
    wout = nc.dram_tensor("wout", [D, D], f32r, kind="ExternalInput").ap()
    qth = nc.dram_tensor("qth", [128, H], f32r, kind="ExternalInput").ap()
    bkdT = nc.dram_tensor("bkdT", [128, 4], dt.float32, kind="ExternalInput").ap()
    bvdr = nc.dram_tensor("bvdr", [128, 512], dt.float32, kind="ExternalInput").ap()
    bvu = nc.dram_tensor("bvu", [DH, H], dt.float32, kind="ExternalInput").ap()
    boutr = nc.dram_tensor("boutr", [1, D], dt.float32, kind="ExternalInput").ap()
    outr = nc.dram_tensor("outr", [1, D], dt.float32, kind="ExternalOutput").ap()

    ND = D // 128        # 16
    NL = L // 128        # 4
    NKB = TOPK // 256    # 8 half-k-blocks (256 wide)
    NKT = TOPK // 128    # 16 k-tiles

    with tile.TileContext(nc) as tc:
        with tc.tile_pool(name="top", bufs=1) as top:
            bkd = top.tile([128, 4], dt.float32)
            nc.sync.dma_start(bkd[:], bkdT)
            bvdrep = top.tile([128, 512], dt.float32)
            nc.sync.dma_start(bvdrep[:], bvdr)
            qthh = top.tile([128, H], f32r)
            nc.sync.dma_start(qthh[:], qth)
            ident = top.tile([128, 128], dt.float32)
            masks.make_identity(nc, ident[:])
            ksT = top.tile([128, NL * TOPK], f32r)         # [l, k] 4 MiB
            vs = top.tile([128, NKT * L], f32r)            # [k, l] 4 MiB

            # ---- phase KV: K_selT (layout B) + V_sel (layout A)
            with (
                tc.tile_pool(name="wkvp", bufs=1) as wkvp,
                tc.tile_pool(name="xpool", bufs=2) as xpool,
                tc.tile_pool(name="kps", bufs=3, space="PSUM") as kps,
            ):
                wkv = wkvp.tile([128, ND * 2 * L], f32r)
                for c in range(ND):
                    nc.sync.dma_start(wkv[:, c * 2 * L:(c + 1) * 2 * L],
                                      wkvd[c * 128:(c + 1) * 128, :])
                for kb in range(NKB):
                    slab = xpool.tile([128, ND * 256], f32r)
                    for c in range(ND):
                        nc.sync.dma_start(
                            slab[:, c * 256:(c + 1) * 256],
                            xselT[c * 128:(c + 1) * 128,
                                  kb * 256:(kb + 1) * 256])
                    for lt in range(NL):
                        pk = kps.tile([128, 256], dt.float32, tag="pk")
                        for d in range(ND):
                            nc.tensor.matmul(
                                pk[:],
                                wkv[:, d * 2 * L + lt * 128:
                                    d * 2 * L + lt * 128 + 128],
                                slab[:, d * 256:(d + 1) * 256],
                                start=(d == 0), stop=(d == ND - 1))
                        nc.vector.tensor_scalar_add(
                            ksT[:, lt * TOPK + kb * 256:
                                lt * TOPK + kb * 256 + 256],
                            pk[:], bkd[:, lt:lt + 1])
                    for kt in range(2):
                        t = kb * 2 + kt
                        pv = kps.tile([128, 512], dt.float32, tag="pv")
                        for d in range(ND):
                            nc.tensor.matmul(
                                pv[:],
                                slab[:, d * 256 + kt * 128:
                                     d * 256 + kt * 128 + 128],
                                wkv[:, d * 2 * L + L: d * 2 * L + 2 * L],
                                start=(d == 0), stop=(d == ND - 1))
                        nc.vector.tensor_add(
                            vs[:, t * L:(t + 1) * L], pv[:], bvdrep[:])

            # ---- qhT[l, h] = sum_dh Wk_upT[hd, l] * qT_heads[hd, h] per head
            # f32r ISA requires even moving-N: compute [l-tile, 16] blocks
            # against ALL heads (rhs = qthh [128,16]) and extract column h.
            qhT = top.tile([128, NL * H], f32r)
            with (
                tc.tile_pool(name="wkp", bufs=2) as wkp,
                tc.tile_pool(name="qps", bufs=2, space="PSUM") as qps,
            ):
                for h in range(H):
                    wkb = wkp.tile([128, L], f32r, tag="wkb")
                    nc.sync.dma_start(wkb[:], wktup[h * 128:(h + 1) * 128, :])
                    for lc in range(NL):
                        pqh = qps.tile([128, H], dt.float32, tag="pqh")
                        nc.tensor.matmul(
                            pqh[:],
                            wkb[:, lc * 128:(lc + 1) * 128],
                            qthh[:],
                            start=True, stop=True)
                        nc.scalar.copy(
                            qhT[:, lc * H + h: lc * H + h + 1],
                            pqh[:, h:h + 1])

            # ---- logits + softmax
            attn2 = top.tile([H, TOPK], dt.float32)
            with (
                tc.tile_pool(name="lpool", bufs=1, space="PSUM") as lpool,
                tc.tile_pool(name="smx", bufs=1) as smx,
            ):
                lps = lpool.tile([128, TOPK], dt.float32)
                for lc in range(NL):
                    for nb in range(4):
                        nc.tensor.matmul(
                            lps[:H, nb * 512:(nb + 1) * 512],
                            qhT[:, lc * H:(lc + 1) * H],
                            ksT[:, lc * TOPK + nb * 512:
                                lc * TOPK + nb * 512 + 512],
                            start=(lc == 0), stop=(lc == NL - 1))
                mx = smx.tile([H, 1], dt.float32)
                nc.vector.reduce_max(mx[:], lps[:H, :],
                                     axis=mybir.AxisListType.X)
                nmx = smx.tile([H, 1], dt.float32)
                nc.vector.tensor_scalar_mul(nmx[:], mx[:], -RSQ)
                attn = smx.tile([H, TOPK], dt.float32)
                den = smx.tile([H, 1], dt.float32)
                nc.scalar.activation(attn[:], lps[:H, :],
                                     mybir.ActivationFunctionType.Exp,
                                     bias=nmx[:], scale=RSQ, accum_out=den[:])
                rden = smx.tile([H, 1], dt.float32)
                nc.vector.reciprocal(rden[:], den[:])
                nc.vector.tensor_scalar_mul(attn2[:], attn[:], rden[:])

            # ---- attnT, m, o, out
            with (
                tc.tile_pool(name="wvp", bufs=2) as wvp,
                tc.tile_pool(name="wop", bufs=3) as wop,
                tc.tile_pool(name="tp", bufs=2, space="PSUM") as tp,
                tc.tile_pool(name="ap", bufs=1, space="PSUM") as ap,
            ):
                attnT = top.tile([128, NKT * H], f32r)
                for t in range(NKT):
                    pt = tp.tile([128, H], dt.float32, tag="tp")
                    nc.tensor.matmul(pt[:], attn2[:, t * 128:(t + 1) * 128],
                                     ident[:H, :H], is_transpose=True)
                    nc.scalar.copy(attnT[:, t * H:(t + 1) * H], pt[:])

                mps = ap.tile([H, L], dt.float32, tag="acc")
                for t in range(NKT):
                    nc.tensor.matmul(mps[:], attnT[:, t * H:(t + 1) * H],
                                     vs[:, t * L:(t + 1) * L],
                                     start=(t == 0), stop=(t == NKT - 1))
                m_sb = top.tile([H, L], dt.float32)
                nc.scalar.copy(m_sb[:], mps[:])
                mT = top.tile([128, NL * H], f32r)
                for lc in range(NL):
                    pmt = tp.tile([128, H], dt.float32, tag="tp")
                    nc.tensor.matmul(pmt[:], m_sb[:, lc * 128:(lc + 1) * 128],
                                     ident[:H, :H], is_transpose=True)
                    nc.scalar.copy(mT[:, lc * H:(lc + 1) * H], pmt[:])

                # oT[dh, h] = sum_lc Wv_up[lc-chunk, h-block].T @ mT[:, lc, h]
                # even-N fix: rhs = all-head mT chunk [128, 16]; accumulate
                # over lc per h (col h valid, others garbage), extract col h.
                wv4 = wvp.tile([128, NL * D], f32r, tag="wv4")
                for lc in range(NL):
                    nc.sync.dma_start(wv4[:, lc * D:(lc + 1) * D],
                                      wvup[lc * 128:(lc + 1) * 128, :])
                bvui = wvp.tile([128, H], dt.float32, tag="bvui")
                nc.sync.dma_start(bvui[:], bvu)
                # oTz: even columns hold oT (+bv_up), odd columns zero, so the
                # out-proj can use even-width [128, 2] stationary slices.
                oTz = wvp.tile([128, 2 * H], f32r, tag="oTz")
                zf = wvp.tile([128, 2 * H], dt.float32, tag="zf")
                nc.vector.memset(zf[:], 0.0)
                nc.vector.tensor_copy(oTz[:], zf[:])
                for h in range(H):
                    poh = ap.tile([128, H], dt.float32, tag="acc2")
                    for lc in range(NL):
                        nc.tensor.matmul(
                            poh[:],
                            wv4[:, lc * D + h * DH: lc * D + (h + 1) * DH],
                            mT[:, lc * H:(lc + 1) * H],
                            start=(lc == 0), stop=(lc == NL - 1))
                    nc.vector.tensor_add(
                        oTz[:, 2 * h:2 * h + 1], poh[:, h:h + 1],
                        bvui[:, h:h + 1])

                # out = o_flat @ Wout + bout
                bouti = wvp.tile([1, D], dt.float32, tag="bouti")
                nc.sync.dma_start(bouti[:], boutr)
                out_sb = wvp.tile([1, D], dt.float32, tag="out_sb")
                for nb in range(4):
                    # M=2 (even) stationary: col 0 real o-chunk, col 1 zeros
                    pout = tp.tile([2, 512], dt.float32, tag="tp")
                    for dc in range(ND):
                        wob = wop.tile([128, 512], f32r, tag="wob")
                        nc.sync.dma_start(
                            wob[:], wout[dc * 128:(dc + 1) * 128,
                                         nb * 512:(nb + 1) * 512])
                        nc.tensor.matmul(pout[:], oTz[:, 2 * dc:2 * dc + 2],
                                         wob[:],
                                         start=(dc == 0), stop=(dc == ND - 1))
                    nc.vector.tensor_add(
                        out_sb[:, nb * 512:(nb + 1) * 512], pout[:1, :],
                        bouti[:, nb * 512:(nb + 1) * 512])
                nc.sync.dma_start(outr, out_sb[:])
    nc.compile()
    return nc


# ---------------------------------------------------------------- timing
def time_launch(nc, in_maps, iters=20):
    """Measure per-execution HW time of a compiled launch: build the sharded
    PJRT executable once, keep inputs device-resident, pipeline `iters`
    executions and average. Donated zero output buffers are refreshed per
    call (tiny)."""
    import time as _time
    import jax
    from jax.sharding import Mesh, PartitionSpec, NamedSharding
    from jax.experimental.shard_map import shard_map
    from concourse import bass2jax

    bass2jax.install_neuronx_cc_hook()
    pname = nc.partition_id_tensor.name if nc.partition_id_tensor else None
    in_names, out_names, out_avals = [], [], []
    for alloc in nc.m.functions[0].allocations:
        if not isinstance(alloc, mybir.MemoryLocationSet):
            continue
        name = alloc.memorylocations[0].name
        if alloc.kind == "ExternalInput":
            if name != pname:
                in_names.append(name)
        elif alloc.kind == "ExternalOutput":
            out_names.append(name)
            out_avals.append(jax.core.ShapedArray(
                tuple(alloc.tensor_shape), mybir.dt.np(alloc.dtype)))
    n_params = len(in_names)
    all_in = in_names + out_names
    if pname is not None:
        all_in = all_in + [pname]
    donate = tuple(range(n_params, n_params + len(out_names)))

    def _body(*args):
        operands = list(args)
        if pname is not None:
            operands.append(bass2jax.partition_id_tensor())
        outs = bass2jax._bass_exec_p.bind(
            *operands, out_avals=tuple(out_avals), in_names=tuple(all_in),
            out_names=tuple(out_names), lowering_input_output_aliases=(),
            sim_require_finite=True, sim_require_nnan=True, nc=nc)
        return tuple(outs)

    n = len(in_maps)
    devices = jax.devices()[:n]
    mesh = Mesh(np.asarray(devices), ("core",))
    fn = jax.jit(
        shard_map(_body, mesh=mesh,
                  in_specs=(PartitionSpec("core"),) * (n_params + len(out_names)),
                  out_specs=(PartitionSpec("core"),) * len(out_names),
                  check_rep=False),
        donate_argnums=donate, keep_unused=True)
    sh = NamedSharding(mesh, PartitionSpec("core"))
    concat_in = [
        jax.device_put(
            np.concatenate([np.asarray(m[name]) for m in in_maps], axis=0), sh)
        for name in in_names]

    def zeros():
        return [jax.device_put(
            np.zeros((n * av.shape[0], *av.shape[1:]), av.dtype), sh)
            for av in out_avals]

    out = fn(*concat_in, *zeros())
    jax.block_until_ready(out)
    zs = [zeros() for _ in range(iters)]
    jax.block_until_ready(zs)
    t0 = _time.perf_counter()
    outs = [fn(*concat_in, *z) for z in zs]
    jax.block_until_ready(outs)
    t1 = _time.perf_counter()
    return (t1 - t0) / iters * 1e9


def model_time(nc):
    """Cost-model (TimelineSim) estimate in ns for one core."""
    from concourse.timeline_sim import TimelineSim
    return TimelineSim(nc).simulate()


def time_launch_chained(nc, in_maps, chains=(1, 17), reps=5):
    """True HW exec: run N back-to-back bass_exec calls inside ONE jit
    (single dispatch), at two chain depths; slope = per-exec time."""
    import time as _time
    import jax
    from jax.sharding import Mesh, PartitionSpec, NamedSharding
    from jax.experimental.shard_map import shard_map
    from concourse import bass2jax

    bass2jax.install_neuronx_cc_hook()
    pname = nc.partition_id_tensor.name if nc.partition_id_tensor else None
    in_names, out_names, out_avals = [], [], []
    for alloc in nc.m.functions[0].allocations:
        if not isinstance(alloc, mybir.MemoryLocationSet):
            continue
        name = alloc.memorylocations[0].name
        if alloc.kind == "ExternalInput":
            if name != pname:
                in_names.append(name)
        elif alloc.kind == "ExternalOutput":
            out_names.append(name)
            out_avals.append(jax.core.ShapedArray(
                tuple(alloc.tensor_shape), mybir.dt.np(alloc.dtype)))
    n_params = len(in_names)
    n_outs = len(out_names)
    all_in = in_names + out_names
    if pname is not None:
        all_in = all_in + [pname]

    def _body(*args):
        operands = list(args)
        if pname is not None:
            operands.append(bass2jax.partition_id_tensor())
        return tuple(bass2jax._bass_exec_p.bind(
            *operands, out_avals=tuple(out_avals), in_names=tuple(all_in),
            out_names=tuple(out_names), lowering_input_output_aliases=(),
            sim_require_finite=True, sim_require_nnan=True, nc=nc))

    n = len(in_maps)
    devices = jax.devices()[:n]
    mesh = Mesh(np.asarray(devices), ("core",))
    sh = NamedSharding(mesh, PartitionSpec("core"))
    concat_in = [
        jax.device_put(
            np.concatenate([np.asarray(m[name]) for m in in_maps], axis=0), sh)
        for name in in_names]
    zero_np = [np.zeros((n * av.shape[0], *av.shape[1:]), av.dtype)
               for av in out_avals]

    times = {}
    for K in chains:
        def _chain(*flat):
            # every call uses the SAME parameter list (hook requires each
            # bass_exec's operands to be params 0..N-1); effectful calls
            # are neither CSE'd nor DCE'd, and run serially per device.
            out = None
            for _ in range(K):
                out = _body(*flat)
            return out

        fn = jax.jit(
            shard_map(_chain, mesh=mesh,
                      in_specs=(PartitionSpec("core"),) * (n_params + n_outs),
                      out_specs=(PartitionSpec("core"),) * n_outs,
                      check_rep=False),
            keep_unused=True)

        zs = [jax.device_put(z, sh) for z in zero_np]
        out = fn(*concat_in, *zs)
        jax.block_until_ready(out)
        best = float("inf")
        for _ in range(reps):
            t0 = _time.perf_counter()
            out = fn(*concat_in, *zs)
            jax.block_until_ready(out)
            best = min(best, _time.perf_counter() - t0)
        times[K] = best
    k0, k1 = chains
    return (times[k1] - times[k0]) / (k1 - k0) * 1e9, times


def _q8j(a):
    import jax.numpy as jnp
    return jnp.asarray(a).astype(jnp.float8_e4m3fn).astype(jnp.float32)


def kernel(**inputs):
    import jax
    import jax.numpy as jnp
    cpu = jax.devices("cpu")[0]

    x = np.ascontiguousarray(np.asarray(inputs["x"], dtype=np.float32))
    Wq = np.asarray(inputs["Wq"], dtype=np.float32)
    bq = np.asarray(inputs["bq"], dtype=np.float32)
    Wkv_down = np.asarray(inputs["Wkv_down"], dtype=np.float32)
    bkv_down = np.asarray(inputs["bkv_down"], dtype=np.float32)
    Wq_down = np.asarray(inputs["Wq_down"], dtype=np.float32)
    bq_down = np.asarray(inputs["bq_down"], dtype=np.float32)
    Wkv_up = np.asarray(inputs["Wkv_up"], dtype=np.float32)
    bkv_up = np.asarray(inputs["bkv_up"], dtype=np.float32)
    Wout = np.asarray(inputs["Wout"], dtype=np.float32)
    bout = np.asarray(inputs["bout"], dtype=np.float32)
    k = int(np.asarray(inputs["top_k"]))
    assert k == TOPK, f"kernel hardcoded for top_k={TOPK}, got {k}"

    if "l1" not in _STATE:
        _STATE["l1"] = _build_l1()
    if "l2" not in _STATE:
        _STATE["l2"] = _build_l2()

    trace = False  # NTFF profiling hook unavailable under this axon client

    q_last = x[:, -1, :]                                   # [B, D]
    with jax.default_device(cpu):
        # bit-exact replication of the reference's fp8 indexer query + q
        q_idx = np.asarray(_q8j(q_last) @ _q8j(Wq_down) + _q8j(bq_down))
        q = np.asarray(jnp.asarray(q_last) @ jnp.asarray(Wq)) + bq

    # ---------------- launch 1: noisy full-S scores
    wdk8 = np.ascontiguousarray(Wkv_down[:, :L]).astype(BF16)
    qxT8 = np.ascontiguousarray(q_idx.T).astype(BF16)      # [L, B]
    bkdT = np.ascontiguousarray(bkv_down[:L].reshape(4, 128).T)
    in1 = []
    for c in range(NCORES):
        in1.append({
            "xT8": np.ascontiguousarray(x[c].T).astype(BF16),
            "wdk8": wdk8,
            "qxT8": qxT8,
            "bkdT": bkdT,
        })
    LAST_IN1.clear(); LAST_IN1.extend(in1)
    r1 = run_bass_kernel_spmd(_STATE["l1"], in1, list(range(NCORES)),
                              trace=trace)
    LAST_EXEC["l1"] = r1
    s_noisy = np.stack([r1.results[c]["scores"][c] for c in range(NCORES)])

    # ---------------- host: exact top-k set via band rescore (bit-exact)
    sel_all = []
    with jax.default_device(cpu):
        jWdk = jnp.asarray(Wkv_down[:, :L])
        jbkd = jnp.asarray(bkv_down[:L])
        for b in range(B):
            order = np.argsort(-np.maximum(s_noisy[b], 0.0), kind="stable")
            certain = order[:k - MARGIN]
            band = order[k - MARGIN:k + MARGIN]
            Kb = jnp.asarray(x[b][band]) @ jWdk + jbkd
            sb = np.asarray(jnp.einsum(
                "l,sl->s", jnp.asarray(q_idx[b]),
                Kb.astype(jnp.float8_e4m3fn).astype(jnp.float32)))
            sb = np.maximum(sb, 0.0)
            pick = band[np.argsort(-sb, kind="stable")[:k - len(certain)]]
            sel_all.append(np.concatenate([certain, pick]))

    # ---------------- launch 2: attention over the selected set
    wktup = np.ascontiguousarray(Wkv_up[:, :D].T)          # [D, L]
    wvup = np.ascontiguousarray(Wkv_up[:, D:])             # [L, D]
    bvdr = np.ascontiguousarray(
        np.broadcast_to(bkv_down[L:], (128, 512))).astype(np.float32)
    bvu = np.ascontiguousarray(bkv_up[D:].reshape(H, DH).T)
    boutr = np.ascontiguousarray(bout.reshape(1, D))
    in2 = []
    for c in range(NCORES):
        in2.append({
            "xselT": np.ascontiguousarray(x[c][sel_all[c]].T),
            "wkvd": Wkv_down,
            "wktup": wktup,
            "wvup": wvup,
            "wout": Wout,
            "qth": np.ascontiguousarray(q[c].reshape(H, DH).T),
            "bkdT": bkdT,
            "bvdr": bvdr,
            "bvu": bvu,
            "boutr": boutr,
        })
    LAST_IN2.clear(); LAST_IN2.extend(in2)
    r2 = run_bass_kernel_spmd(_STATE["l2"], in2, list(range(NCORES)),
                              trace=trace)
    LAST_EXEC["l2"] = r2
    out = np.stack([r2.results[c]["outr"][0] for c in range(NCORES)])
    return out.astype(np.float32)
